# revision 32
# baseline (speedup 1.0000x reference)
"""AttGRU cell on 8 TRN2 NeuronCores.

Math (per reference):
    agg = einsum('ij,bj->bi', adj, x)                  # [B, N]
    r   = sigmoid(agg + h @ W_hr.T + b_hr)
    z   = sigmoid(agg + h @ W_hz.T + b_hz)
    n   = tanh(agg + r * (h @ W_hn.T + b_hn))
    out = (1 - z) * n + z * h
B=8, N=4096. Memory-bound: streaming the four [N, N] matrices dominates.

Sharding: row-shard adj/W_* over 8 cores (512 output features per core),
replicate x/h (tiny). Each core computes its 512 output columns; the host
concatenates. No collectives.

Design (all-fp8 stream, 8 MB/core, DMA-bound end to end):
- Every weight matrix ships as 1-byte fp8, formats sized to the error
  each term can carry (l2 rel err ~1.55e-2 vs the 2e-2 gate):
  * adj (x4096), W_hr (x64), W_hz (x64): e4m3, consumed by DoubleRow
    matmuls (2 fp8 weights/PE cell -> a 1MB slab in 8 matmuls) with fp8
    x/h stationaries. r/z pre-activation errors are attenuated by
    sigmoid'; agg is a ~0.01-std additive term.
  * W_hn: e3m4 x128 (4-bit mantissa halves the quantization error; the
    n path sets output accuracy directly), consumed by normal-mode
    matmuls with *bf16* h stationary (mixed-dtype matmul is legal).
- The PE has a ~216ns per-matmul floor, so a normal-mode 1MB slab costs
  3.46us against a ~2.95us DMA slab cadence; DoubleRow slabs cost
  1.73us. With only W_hn in normal mode the PE tracks the stream.
- All 8 slab buffers live in SBUF simultaneously (8 MB of 24 MB): the
  HBM stream never waits on a buffer recycle; a single sync-ring queue
  sustains ~350 GB/s (the per-core share of the HBM stack with all 8
  cores streaming; a second ring adds nothing).
- The PE idles ~11us during the Tile preamble, so its HAM clock gate
  would hold it at 1.2 GHz for the first ~3.4us of real matmuls and any
  >~3us data stall re-throttles it. A memset scratch tile + 8 dummy
  matmuls warm the gate before slab 0 lands, and 4 fillers after each
  DoubleRow slab bridge the DMA cadence so the PE never idles long
  enough to re-throttle.
- Biases enter PSUM via K=1 matmuls (ones[1,B].T @ b[1,S]) as group
  openers; agg folds into the z accumulator via a 64*I f32 matmul.
  tanh(u) = 2*sigmoid(2u)-1 keeps ScalarE on a single activation table.
- Stream order adj -> W_hr -> W_hn -> W_hz: the n epilogue overlaps the
  W_hz slabs and the tail after the last weight byte is just the z
  quarter-chain (sigmoid, mul, add, out-DMA on alternating rings).

Per-core inputs (host-prepared):
  wdr  [6, 128, 16, 512] e4m3 - adj (slabs 0-1) | W_hr (2-3) | W_hz
                                (4-5), row-shard, transposed, chunk-major
  whn8 [2, 128, 8192] e3m4 - W_hn x128, chunk-major
  vtx8p [128, 2, 128] e4m3 - x.T chunk pairs (even plane 0 / odd plane 1)
  vth8p [128, 2, 128] e4m3 - h.T chunk pairs
  vth  [128, 256] bf16 - h.T per chunk (n-gate stationary)
  bvec [1, 1536] bf16  - b_hr*64 | b_hn*128 | b_hz*64 shards
  ones1 [1, 8] bf16, eyez [8, 8] f32 (=64*I), hloc [8, 512] f32
"""

from contextlib import ExitStack

import ml_dtypes
import numpy as np

import concourse.bass as bass
import concourse.tile as tile
from concourse import bacc, mybir
from concourse.bass_utils import run_bass_kernel_spmd
from concourse.tile import add_dep_helper

B = 8
N = 4096
NCORES = 8
S = N // NCORES          # 512 output cols per core
KC = 128                 # contraction chunk (PE partition dim)
NK = N // KC             # 32 chunks per gate
NKP = NK // 2            # 16 chunk pairs (DoubleRow)
CPS = 16                 # chunks per slab ([128, 16, 512] = 1MB)
SLABW = CPS * S          # 8192 flat columns per slab
NSLABS = 8               # adj(2) + W_hr(2) + W_hn(2) + W_hz(2)
WDR_SRC = {0: 0, 1: 1, 2: 2, 3: 3, 6: 4, 7: 5}  # slab -> wdr index
ZQ = S // 4              # (unused) quarter width
ZH = S // 2              # z tail computed in column halves
N_WARM = 12              # dummy matmuls to lift the PE clock gate
N_FILL = 6               # narrow fillers after each fast slab
ADJ_SCALE = 4096.0       # adj pre-scale so e4m3 doesn't flush to zero
WHR_SCALE = 64.0         # W_hr/W_hz pre-scale: N(0,1/64^2) -> N(0,1)
WHN_SCALE = 128.0        # W_hn pre-scale for e3m4 (max normal 15.5)

BF16 = mybir.dt.bfloat16
F32 = mybir.dt.float32
FP8 = mybir.dt.float8e4
FP8E3 = mybir.dt.float8e3
DR = mybir.MatmulPerfMode.DoubleRow

_CACHED_NC = None


def _build():
    nc = bacc.Bacc(
        "TRN2",
        target_bir_lowering=False,
        debug=False,
        num_devices=NCORES,
    )
    wdr = nc.dram_tensor("wdr", [6, KC, CPS, S], FP8, kind="ExternalInput")
    whn8 = nc.dram_tensor("whn8", [2, KC, SLABW], FP8E3, kind="ExternalInput")
    vtx8p = nc.dram_tensor("vtx8p", [KC, 2, NKP * B], FP8, kind="ExternalInput")
    vth8p = nc.dram_tensor("vth8p", [KC, 2, NKP * B], FP8, kind="ExternalInput")
    vth = nc.dram_tensor("vth", [KC, NK * B], BF16, kind="ExternalInput")
    bvec = nc.dram_tensor("bvec", [1, 3 * S], BF16, kind="ExternalInput")
    ones1 = nc.dram_tensor("ones1", [1, B], BF16, kind="ExternalInput")
    hloc = nc.dram_tensor("hloc", [B, S], F32, kind="ExternalInput")
    hloc1 = nc.dram_tensor("hloc1", [B, S], F32, kind="ExternalInput")
    eyez = nc.dram_tensor("eyez", [B, B], F32, kind="ExternalInput")
    out = nc.dram_tensor("out", [B, S], F32, kind="ExternalOutput")

    AF = mybir.ActivationFunctionType
    ALU = mybir.AluOpType

    with tile.TileContext(nc) as tc, ExitStack() as ctx:
        wpool = ctx.enter_context(tc.tile_pool(name="wall", bufs=NSLABS))
        cpool = ctx.enter_context(tc.tile_pool(name="const", bufs=1))
        ppool = ctx.enter_context(tc.tile_pool(name="acc", bufs=1, space="PSUM"))
        epool = ctx.enter_context(tc.tile_pool(name="epi", bufs=1))

        # PE warmup (see module docstring)
        warm_sb = cpool.tile([KC, S], BF16, tag="warm")
        nc.vector.memset(warm_sb[:], 0.0)
        warm_ps = ppool.tile([B, S], F32, tag="warmps")

        def filler(n, w=128):
            # narrow matmuls: enough PE activity to hold the HAM clock
            # gate at 8/8, at ~90ns each instead of 216ns
            for _ in range(n):
                nc.tensor.matmul(
                    warm_ps[:, :w], warm_sb[:, :B], warm_sb[:, :w],
                    start=True, stop=True,
                )

        # the warmup itself needs ~3.4us of continuous PE busy -> full width
        filler(N_WARM, w=S)

        # all consts on gpsimd SWDGE (vtx8p first - the first matmul needs
        # it); the sync ring stays clear so slab 0 issues immediately
        vtx8p_sb = cpool.tile([KC, 2, NKP * B], FP8, tag="vtx8p")
        nc.gpsimd.dma_start(vtx8p_sb[:], vtx8p[:])
        vth8p_sb = cpool.tile([KC, 2, NKP * B], FP8, tag="vth8p")
        nc.gpsimd.dma_start(vth8p_sb[:], vth8p[:])
        vth_sb = cpool.tile([KC, NK * B], BF16, tag="vth")
        nc.gpsimd.dma_start(vth_sb[:], vth[:])
        bvec_sb = cpool.tile([1, 3 * S], BF16, tag="bvec")
        nc.gpsimd.dma_start(bvec_sb[:], bvec[:])
        ones_sb = cpool.tile([1, B], BF16, tag="ones1")
        nc.gpsimd.dma_start(ones_sb[:], ones1[:])
        hloc_sb = cpool.tile([B, S], F32, tag="hloc")
        nc.gpsimd.dma_start(hloc_sb[:], hloc[:])
        hloc1_sb = cpool.tile([B, S], F32, tag="hloc1")
        nc.gpsimd.dma_start(hloc1_sb[:], hloc1[:])
        eyez_sb = cpool.tile([B, B], F32, tag="eyez")
        nc.gpsimd.dma_start(eyez_sb[:], eyez[:])

        acc = [
            ppool.tile([B, S], F32, tag=f"acc{g}", name=f"acc{g}") for g in range(4)
        ]

        # epilogue tiles, declared up front
        s_agg = epool.tile([B, S], F32, tag="sagg")
        t_r = epool.tile([B, S], F32, tag="tr")
        r_t = epool.tile([B, S], F32, tag="r")
        t_n = epool.tile([B, S], F32, tag="tn")
        t_n2 = epool.tile([B, S], F32, tag="tn2")
        sg_t = epool.tile([B, S], F32, tag="sg")
        n_t = epool.tile([B, S], F32, tag="n")
        d_t = epool.tile([B, S], F32, tag="d")
        z_t = epool.tile([B, S], F32, tag="z")
        zd_t = epool.tile([B, S], F32, tag="zd")
        o_t = epool.tile([B, S], F32, tag="o")

        def bias_open(g, lo):
            return nc.tensor.matmul(
                acc[g][:, :], ones_sb[:, :], bvec_sb[:, lo : lo + S],
                start=True, stop=False,
            )

        whn_reduce = [None]

        # the Tile scheduler is free to reorder the independent slab
        # DMAs (and did, pushing a late gate's slab ahead of an earlier
        # gate's); chain them so the queue transfers in consumption order
        last_dma = [None]

        def slab_dma(wa_flat, src_flat, first, last):
            if first:
                splits = (2048, 2048, 2048, 2048)
            elif last:
                splits = (4096, 2048, 2048)
            else:
                splits = (SLABW,)
            c0 = 0
            for w in splits:
                inst = nc.sync.dma_start(
                    wa_flat[:, c0 : c0 + w], src_flat[:, c0 : c0 + w]
                )
                if last_dma[0] is not None:
                    add_dep_helper(
                        inst.ins, last_dma[0].ins, sync=False,
                        reason="stream slabs in consumption order",
                    )
                last_dma[0] = inst
                c0 += w

        def dr_slab(sl, g, vp, opener=None):
            # one DoubleRow slab: DMA + 8 chunk-pair matmuls
            half = sl % 2
            wa = wpool.tile([KC, CPS, S], FP8, tag="wa", name=f"wa{sl}")
            slab_dma(
                wa.rearrange("p c s -> p (c s)"),
                wdr[WDR_SRC[sl]].rearrange("p c s -> p (c s)"),
                first=(sl == 0), last=(sl == NSLABS - 1),
            )
            if opener is not None:
                opener()
            for c in range(0, CPS, 2):
                kp = half * (CPS // 2) + c // 2
                nc.tensor.matmul(
                    acc[g][:, :],
                    vp[:, :, kp * B : (kp + 1) * B],
                    wa[:, c : c + 2, :],
                    start=(g == 0 and kp == 0),
                    stop=(kp == NKP - 1),
                    perf_mode=DR,
                )

        def whn_slab(sl):
            # W_hn: e3m4 moving x bf16 h stationary, normal mode. 16
            # matmuls per slab (~3.5us) against the ~3us DMA cadence -
            # the PE tracks the stream with no fillers needed, and the
            # accumulator closes right after the slab lands, keeping the
            # serial n epilogue off the tail.
            half = sl % 2
            wa = wpool.tile([KC, SLABW], FP8E3, tag="wa", name=f"wa{sl}")
            slab_dma(wa, whn8[half], first=False, last=False)
            for c in range(CPS):
                k = half * CPS + c
                if k == 0:
                    bias_open(2, S)
                nc.tensor.matmul(
                    acc[2][:, :],
                    vth_sb[:, k * B : (k + 1) * B],
                    wa[:, c * S : (c + 1) * S],
                    start=False,
                    stop=(k == NK - 1),
                )

        def z_opener():
            bias_open(3, 2 * S)
            # fold WHR_SCALE*agg into the z accumulator
            nc.tensor.matmul(
                acc[3][:, :], eyez_sb[:, :], s_agg[:, :],
                start=False, stop=False,
            )

        # ---- the stream, in explicit PE/queue order ----
        # slab order: adj(0-1), W_hr(2-3), W_hn(4-5), W_hz(6-7). W_hr
        # streams early so the serial r -> n epilogue chain (~4us of
        # ACT/DVE ops) completes during the W_hz slabs; the tail after
        # the last weight byte is only the z chain. Fillers bridge each
        # slab's ~1.7us of matmuls against the ~3us DMA cadence so HAM
        # never re-throttles.
        dr_slab(0, 0, vtx8p_sb)
        filler(N_FILL)
        dr_slab(1, 0, vtx8p_sb)
        filler(N_FILL)
        nc.vector.tensor_scalar_mul(s_agg[:], acc[0][:, :], 1.0 / ADJ_SCALE)

        # W_hr: slabs 2-3 (DoubleRow); r_t lands ~2/3 into the stream
        dr_slab(2, 1, vth8p_sb, opener=lambda: bias_open(1, 0))
        filler(N_FILL)
        dr_slab(3, 1, vth8p_sb)
        filler(N_FILL)
        nc.vector.scalar_tensor_tensor(
            t_r[:], acc[1][:, :], 1.0 / WHR_SCALE, s_agg[:],
            ALU.mult, ALU.add,
        )
        nc.scalar.activation(r_t[:], t_r[:], AF.Sigmoid)

        # W_hn: slabs 4-5 (normal mode)
        whn_slab(4)
        whn_slab(5)
        # n epilogue: starts the moment acc2 closes, overlapping the
        # W_hz stream. d_t = (h+1) - 2*sigmoid avoids waiting on n_t.
        nc.vector.tensor_mul(t_n[:], acc[2][:, :], r_t[:])
        nc.vector.scalar_tensor_tensor(
            t_n2[:], t_n[:], 1.0 / WHN_SCALE, s_agg[:], ALU.mult, ALU.add
        )
        # tanh(u) = 2*sigmoid(2u) - 1 (ACT on one table)
        nc.scalar.activation(sg_t[:], t_n2[:], AF.Sigmoid, scale=2.0)
        nc.vector.scalar_tensor_tensor(
            d_t[:], sg_t[:], -2.0, hloc1_sb[:], ALU.mult, ALU.add
        )
        nc.vector.tensor_scalar(
            n_t[:], sg_t[:], 2.0, 1.0, ALU.mult, ALU.subtract
        )

        # W_hz: slabs 6-7 (DoubleRow), then the z tail in column halves
        # on independent rings (both out-DMAs dispatch concurrently)
        dr_slab(6, 3, vth8p_sb, opener=z_opener)
        filler(N_FILL)
        dr_slab(7, 3, vth8p_sb)
        for hf in range(2):
            cols = slice(hf * ZH, (hf + 1) * ZH)
            nc.scalar.activation(
                z_t[:, cols], acc[3][:, cols], AF.Sigmoid,
                scale=1.0 / WHR_SCALE,
            )
            nc.vector.tensor_mul(zd_t[:, cols], z_t[:, cols], d_t[:, cols])
            nc.vector.tensor_add(o_t[:, cols], zd_t[:, cols], n_t[:, cols])
            dma_eng = nc.sync if hf == 0 else nc.gpsimd
            dma_eng.dma_start(out[:, cols], o_t[:, cols])

    nc.compile()
    return nc


def _get_nc():
    global _CACHED_NC
    if _CACHED_NC is None:
        _CACHED_NC = _build()
    return _CACHED_NC


def make_in_maps(x, h, adj, W_hr, b_hr, W_hz, b_hz, W_hn, b_hn):
    bf = ml_dtypes.bfloat16
    fp8 = ml_dtypes.float8_e4m3fn
    fp8e3 = ml_dtypes.float8_e3m4
    x = np.asarray(x, np.float32)
    h = np.asarray(h, np.float32)
    adj = np.asarray(adj, np.float32)
    W_hr = np.asarray(W_hr, np.float32)
    W_hz = np.asarray(W_hz, np.float32)
    W_hn = np.asarray(W_hn, np.float32)
    b_hr = np.asarray(b_hr, np.float32)
    b_hz = np.asarray(b_hz, np.float32)
    b_hn = np.asarray(b_hn, np.float32)

    def pack_vt(v):
        # [B, N] -> [KC, NK, B] chunk-major
        return np.ascontiguousarray(v.T.reshape(NK, KC, B).transpose(1, 0, 2))

    def pack_vt_pairs(v):
        # [KC, NK, B] -> [KC, 2, NKP*B]: even chunks plane 0, odd plane 1
        c = pack_vt(v)
        return np.ascontiguousarray(
            c.reshape(KC, NKP, 2, B).transpose(0, 2, 1, 3).reshape(KC, 2, NKP * B)
        )

    vtx8p_packed = pack_vt_pairs(x).astype(fp8)
    vth8p_packed = pack_vt_pairs(h).astype(fp8)
    vth_packed = pack_vt(h).reshape(KC, NK * B).astype(bf)

    def pack_slabs(chunks_2d, nslabs):
        # [N, S] (contraction-major) -> [nslabs, KC, CPS, S]
        return np.ascontiguousarray(
            chunks_2d.reshape(nslabs, CPS, KC, S).transpose(0, 2, 1, 3)
        )

    in_maps = []
    for s in range(NCORES):
        rs, re = s * S, (s + 1) * S
        wdrp = np.concatenate(
            [
                pack_slabs(np.ascontiguousarray(adj[rs:re].T) * ADJ_SCALE, 2),
                pack_slabs(np.ascontiguousarray(W_hr[rs:re].T) * WHR_SCALE, 2),
                pack_slabs(np.ascontiguousarray(W_hz[rs:re].T) * WHR_SCALE, 2),
            ]
        ).astype(fp8)
        whn8p = (
            pack_slabs(np.clip(W_hn[rs:re].T * WHN_SCALE, -15.0, 15.0), 2)
            .reshape(2, KC, SLABW)
            .astype(fp8e3)
        )
        bvecp = np.concatenate(
            [b_hr[rs:re] * WHR_SCALE, b_hn[rs:re] * WHN_SCALE,
             b_hz[rs:re] * WHR_SCALE]
        )[None, :].astype(bf)
        in_maps.append(
            {
                "wdr": wdrp,
                "whn8": whn8p,
                "vtx8p": vtx8p_packed,
                "vth8p": vth8p_packed,
                "vth": vth_packed,
                "bvec": bvecp,
                "ones1": np.ones((1, B), dtype=bf),
                "hloc": np.ascontiguousarray(h[:, rs:re]),
                "hloc1": np.ascontiguousarray(h[:, rs:re] + 1.0),
                "eyez": np.eye(B, dtype=np.float32) * WHR_SCALE,
            }
        )
    return in_maps


def run(in_maps, trace=False, **kw):
    nc = _get_nc()
    return run_bass_kernel_spmd(
        nc, in_maps, core_ids=list(range(NCORES)), trace=trace, **kw
    )


def kernel(x, h, adj, W_hr, b_hr, W_hz, b_hz, W_hn, b_hn):
    in_maps = make_in_maps(x, h, adj, W_hr, b_hr, W_hz, b_hz, W_hn, b_hn)
    res = run(in_maps)
    return np.concatenate(
        [np.asarray(res.results[s]["out"]) for s in range(NCORES)], axis=1
    )


# revision 33
# speedup vs baseline: 1.0113x; 1.0113x over previous
"""AttGRU cell on 8 TRN2 NeuronCores.

Math (per reference):
    agg = einsum('ij,bj->bi', adj, x)                  # [B, N]
    r   = sigmoid(agg + h @ W_hr.T + b_hr)
    z   = sigmoid(agg + h @ W_hz.T + b_hz)
    n   = tanh(agg + r * (h @ W_hn.T + b_hn))
    out = (1 - z) * n + z * h
B=8, N=4096. Memory-bound: streaming the four [N, N] matrices dominates.

Sharding: row-shard adj/W_* over 8 cores (512 output features per core),
replicate x/h (tiny). Each core computes its 512 output columns; the host
concatenates. No collectives.

Design (all-fp8 stream, 8 MB/core vs 16 MB bf16 / 32 MB f32):
- Every weight matrix ships as 1-byte fp8, formats sized to the error
  each term can carry (l2 rel err ~1.55e-2 vs the 2e-2 gate; numpy
  simulation of the quantization reproduces the HW error to 4 digits):
  * adj (x4096), W_hr (x64), W_hz (x64): e4m3, consumed by DoubleRow
    matmuls (2 fp8 weights/PE cell -> a 1MB slab in 8 matmuls) with fp8
    x/h stationaries. r/z pre-activation errors are attenuated by
    sigmoid'; agg is a ~0.01-std additive term.
  * W_hn: e3m4 x128 (4-bit mantissa halves the quantization error; the
    n path sets output accuracy directly), consumed by normal-mode
    matmuls with *bf16* h stationary (mixed-dtype matmul is legal, so
    the h quantization error stays out of the n gate).
- The PE has a ~216ns per-matmul issue floor at N=512, independent of
  dtype and of halving N: a normal-mode 1MB slab costs 3.46us against a
  ~3us DMA slab cadence, a DoubleRow slab 1.73us. Only W_hn runs normal
  mode, so the stream is DMA-bound end to end.
- All 8 slab buffers live in SBUF simultaneously (8 MB of 24 MB): the
  HBM stream never waits on a buffer recycle. A single sync-ring queue
  sustains ~310-350 GB/s - the per-core share of the HBM stack with all
  8 cores streaming; a second ring adds no total bandwidth (measured).
- The Tile scheduler may reorder independent slab DMAs; ordering-only
  dep edges chain them so the queue transfers in consumption order.
- HAM management: the PE idles ~11us during the Tile preamble, so its
  clock gate would hold 1.2 GHz for the first ~3.4us of matmuls, and
  any >~3us idle mid-stream re-throttles it (each costs ~2x on every
  matmul for ~3.4us). Twelve full-width warmup matmuls on a memset
  scratch tile lift the gate before slab 0 lands; six narrow (N=128,
  ~90ns) fillers after each DoubleRow slab bridge the DMA cadence.
- Stream order adj -> W_hr -> W_hn -> W_hz: r_t lands mid-stream, so
  the serial r->n epilogue chain (~3us of ACT/DVE ops on [8,512] tiles)
  overlaps the W_hz slabs; d_t = (h+1) - 2*sigmoid skips a dependency
  on n_t. The tail after the last weight byte is only the z chain:
  sigmoid/mul/add in column halves, out-DMA halves on the sync and
  gpsimd rings so both dispatch concurrently.
- Biases enter PSUM via K=1 matmuls (ones[1,B].T @ b[1,S]) as group
  openers; agg folds into the z accumulator via a 64*I f32 matmul.
  tanh(u) = 2*sigmoid(2u) - 1 keeps ScalarE on a single activation
  table.

Per-core inputs (host-prepared):
  wdr  [6, 128, 16, 512] e4m3 - adj (slabs 0-1) | W_hr (2-3) | W_hz
                                (4-5), row-shard, transposed, chunk-major
  whn8 [2, 128, 8192] e3m4 - W_hn x128, chunk-major
  vtx8p [128, 2, 128] e4m3 - x.T chunk pairs (even plane 0 / odd plane 1)
  vth8p [128, 2, 128] e4m3 - h.T chunk pairs
  vth  [128, 256] bf16 - h.T per chunk (n-gate stationary)
  bvec [1, 1536] bf16  - b_hr*64 | b_hn*128 | b_hz*64 shards
  ones1 [1, 8] bf16, eyez [8, 8] f32 (=64*I)
  hloc/hloc1 [8, 512] f32 - h columns, and h+1 for the d_t shortcut
"""

from contextlib import ExitStack

import ml_dtypes
import numpy as np

import concourse.bass as bass
import concourse.tile as tile
from concourse import bacc, mybir
from concourse.bass_utils import run_bass_kernel_spmd
from concourse.tile import add_dep_helper

B = 8
N = 4096
NCORES = 8
S = N // NCORES          # 512 output cols per core
KC = 128                 # contraction chunk (PE partition dim)
NK = N // KC             # 32 chunks per gate
NKP = NK // 2            # 16 chunk pairs (DoubleRow)
CPS = 16                 # chunks per slab ([128, 16, 512] = 1MB)
SLABW = CPS * S          # 8192 flat columns per slab
NSLABS = 8               # adj(2) + W_hr(2) + W_hn(2) + W_hz(2)
WDR_SRC = {0: 0, 1: 1, 2: 2, 3: 3, 6: 4, 7: 5}  # slab -> wdr index
ZH = S // 2              # z tail computed in column halves
N_WARM = 12              # dummy matmuls to lift the PE clock gate
N_FILL = 6               # narrow fillers after each fast slab
ADJ_SCALE = 4096.0       # adj pre-scale so e4m3 doesn't flush to zero
WHR_SCALE = 64.0         # W_hr/W_hz pre-scale: N(0,1/64^2) -> N(0,1)
WHN_SCALE = 128.0        # W_hn pre-scale for e3m4 (max normal 15.5)

BF16 = mybir.dt.bfloat16
F32 = mybir.dt.float32
FP8 = mybir.dt.float8e4
FP8E3 = mybir.dt.float8e3
DR = mybir.MatmulPerfMode.DoubleRow

_CACHED_NC = None


def _build():
    nc = bacc.Bacc(
        "TRN2",
        target_bir_lowering=False,
        debug=False,
        num_devices=NCORES,
    )
    wdr = nc.dram_tensor("wdr", [6, KC, CPS, S], FP8, kind="ExternalInput")
    whn8 = nc.dram_tensor("whn8", [2, KC, SLABW], FP8E3, kind="ExternalInput")
    vtx8p = nc.dram_tensor("vtx8p", [KC, 2, NKP * B], FP8, kind="ExternalInput")
    vth8p = nc.dram_tensor("vth8p", [KC, 2, NKP * B], FP8, kind="ExternalInput")
    vth = nc.dram_tensor("vth", [KC, NK * B], BF16, kind="ExternalInput")
    bvec = nc.dram_tensor("bvec", [1, 3 * S], BF16, kind="ExternalInput")
    ones1 = nc.dram_tensor("ones1", [1, B], BF16, kind="ExternalInput")
    hloc = nc.dram_tensor("hloc", [B, S], F32, kind="ExternalInput")
    hloc1 = nc.dram_tensor("hloc1", [B, S], F32, kind="ExternalInput")
    eyez = nc.dram_tensor("eyez", [B, B], F32, kind="ExternalInput")
    out = nc.dram_tensor("out", [B, S], F32, kind="ExternalOutput")

    AF = mybir.ActivationFunctionType
    ALU = mybir.AluOpType

    with tile.TileContext(nc) as tc, ExitStack() as ctx:
        wpool = ctx.enter_context(tc.tile_pool(name="wall", bufs=NSLABS))
        cpool = ctx.enter_context(tc.tile_pool(name="const", bufs=1))
        ppool = ctx.enter_context(tc.tile_pool(name="acc", bufs=1, space="PSUM"))
        epool = ctx.enter_context(tc.tile_pool(name="epi", bufs=1))

        # PE warmup (see module docstring)
        warm_sb = cpool.tile([KC, S], BF16, tag="warm")
        nc.vector.memset(warm_sb[:], 0.0)
        warm_ps = ppool.tile([B, S], F32, tag="warmps")

        def filler(n, w=128):
            # narrow matmuls: enough PE activity to hold the HAM clock
            # gate at 8/8, at ~90ns each instead of 216ns
            for _ in range(n):
                nc.tensor.matmul(
                    warm_ps[:, :w], warm_sb[:, :B], warm_sb[:, :w],
                    start=True, stop=True,
                )

        # the warmup itself needs ~3.4us of continuous PE busy -> full width
        filler(N_WARM, w=S)

        # all consts on gpsimd SWDGE (vtx8p first - the first matmul needs
        # it); the sync ring stays clear so slab 0 issues immediately
        vtx8p_sb = cpool.tile([KC, 2, NKP * B], FP8, tag="vtx8p")
        nc.gpsimd.dma_start(vtx8p_sb[:], vtx8p[:])
        vth8p_sb = cpool.tile([KC, 2, NKP * B], FP8, tag="vth8p")
        nc.gpsimd.dma_start(vth8p_sb[:], vth8p[:])
        vth_sb = cpool.tile([KC, NK * B], BF16, tag="vth")
        nc.gpsimd.dma_start(vth_sb[:], vth[:])
        bvec_sb = cpool.tile([1, 3 * S], BF16, tag="bvec")
        nc.gpsimd.dma_start(bvec_sb[:], bvec[:])
        ones_sb = cpool.tile([1, B], BF16, tag="ones1")
        nc.gpsimd.dma_start(ones_sb[:], ones1[:])
        hloc_sb = cpool.tile([B, S], F32, tag="hloc")
        nc.gpsimd.dma_start(hloc_sb[:], hloc[:])
        hloc1_sb = cpool.tile([B, S], F32, tag="hloc1")
        nc.gpsimd.dma_start(hloc1_sb[:], hloc1[:])
        eyez_sb = cpool.tile([B, B], F32, tag="eyez")
        nc.gpsimd.dma_start(eyez_sb[:], eyez[:])

        acc = [
            ppool.tile([B, S], F32, tag=f"acc{g}", name=f"acc{g}") for g in range(4)
        ]

        # epilogue tiles, declared up front
        s_agg = epool.tile([B, S], F32, tag="sagg")
        t_r = epool.tile([B, S], F32, tag="tr")
        r_t = epool.tile([B, S], F32, tag="r")
        t_n = epool.tile([B, S], F32, tag="tn")
        t_n2 = epool.tile([B, S], F32, tag="tn2")
        sg_t = epool.tile([B, S], F32, tag="sg")
        n_t = epool.tile([B, S], F32, tag="n")
        d_t = epool.tile([B, S], F32, tag="d")
        z_t = epool.tile([B, S], F32, tag="z")
        zd_t = epool.tile([B, S], F32, tag="zd")
        o_t = epool.tile([B, S], F32, tag="o")

        def bias_open(g, lo):
            return nc.tensor.matmul(
                acc[g][:, :], ones_sb[:, :], bvec_sb[:, lo : lo + S],
                start=True, stop=False,
            )

        whn_reduce = [None]

        # the Tile scheduler is free to reorder the independent slab
        # DMAs (and did, pushing a late gate's slab ahead of an earlier
        # gate's); chain them so the queue transfers in consumption order
        last_dma = [None]

        def slab_dma(wa_flat, src_flat, first, last):
            if first:
                splits = (2048, 2048, 2048, 2048)
            elif last:
                splits = (4096, 2048, 2048)
            else:
                splits = (SLABW,)
            c0 = 0
            for w in splits:
                inst = nc.sync.dma_start(
                    wa_flat[:, c0 : c0 + w], src_flat[:, c0 : c0 + w]
                )
                if last_dma[0] is not None:
                    add_dep_helper(
                        inst.ins, last_dma[0].ins, sync=False,
                        reason="stream slabs in consumption order",
                    )
                last_dma[0] = inst
                c0 += w

        def dr_slab(sl, g, vp, opener=None):
            # one DoubleRow slab: DMA + 8 chunk-pair matmuls
            half = sl % 2
            wa = wpool.tile([KC, CPS, S], FP8, tag="wa", name=f"wa{sl}")
            slab_dma(
                wa.rearrange("p c s -> p (c s)"),
                wdr[WDR_SRC[sl]].rearrange("p c s -> p (c s)"),
                first=(sl == 0), last=(sl == NSLABS - 1),
            )
            if opener is not None:
                opener()
            for c in range(0, CPS, 2):
                kp = half * (CPS // 2) + c // 2
                nc.tensor.matmul(
                    acc[g][:, :],
                    vp[:, :, kp * B : (kp + 1) * B],
                    wa[:, c : c + 2, :],
                    start=(g == 0 and kp == 0),
                    stop=(kp == NKP - 1),
                    perf_mode=DR,
                )

        def whn_slab(sl):
            # W_hn: e3m4 moving x bf16 h stationary, normal mode. 16
            # matmuls per slab (~3.5us) against the ~3us DMA cadence -
            # the PE tracks the stream with no fillers needed, and the
            # accumulator closes right after the slab lands, keeping the
            # serial n epilogue off the tail.
            half = sl % 2
            wa = wpool.tile([KC, SLABW], FP8E3, tag="wa", name=f"wa{sl}")
            slab_dma(wa, whn8[half], first=False, last=False)
            for c in range(CPS):
                k = half * CPS + c
                if k == 0:
                    bias_open(2, S)
                nc.tensor.matmul(
                    acc[2][:, :],
                    vth_sb[:, k * B : (k + 1) * B],
                    wa[:, c * S : (c + 1) * S],
                    start=False,
                    stop=(k == NK - 1),
                )

        def z_opener():
            bias_open(3, 2 * S)
            # fold WHR_SCALE*agg into the z accumulator
            nc.tensor.matmul(
                acc[3][:, :], eyez_sb[:, :], s_agg[:, :],
                start=False, stop=False,
            )

        # ---- the stream, in explicit PE/queue order ----
        # slab order: adj(0-1), W_hr(2-3), W_hn(4-5), W_hz(6-7). W_hr
        # streams early so the serial r -> n epilogue chain (~4us of
        # ACT/DVE ops) completes during the W_hz slabs; the tail after
        # the last weight byte is only the z chain. Fillers bridge each
        # slab's ~1.7us of matmuls against the ~3us DMA cadence so HAM
        # never re-throttles.
        dr_slab(0, 0, vtx8p_sb)
        filler(N_FILL)
        dr_slab(1, 0, vtx8p_sb)
        filler(N_FILL)
        nc.vector.tensor_scalar_mul(s_agg[:], acc[0][:, :], 1.0 / ADJ_SCALE)

        # W_hr: slabs 2-3 (DoubleRow); r_t lands ~2/3 into the stream
        dr_slab(2, 1, vth8p_sb, opener=lambda: bias_open(1, 0))
        filler(N_FILL)
        dr_slab(3, 1, vth8p_sb)
        filler(N_FILL)
        nc.vector.scalar_tensor_tensor(
            t_r[:], acc[1][:, :], 1.0 / WHR_SCALE, s_agg[:],
            ALU.mult, ALU.add,
        )
        nc.scalar.activation(r_t[:], t_r[:], AF.Sigmoid)

        # W_hn: slabs 4-5 (normal mode)
        whn_slab(4)
        whn_slab(5)
        # n epilogue: starts the moment acc2 closes, overlapping the
        # W_hz stream. d_t = (h+1) - 2*sigmoid avoids waiting on n_t.
        nc.vector.tensor_mul(t_n[:], acc[2][:, :], r_t[:])
        nc.vector.scalar_tensor_tensor(
            t_n2[:], t_n[:], 1.0 / WHN_SCALE, s_agg[:], ALU.mult, ALU.add
        )
        # tanh(u) = 2*sigmoid(2u) - 1 (ACT on one table)
        nc.scalar.activation(sg_t[:], t_n2[:], AF.Sigmoid, scale=2.0)
        nc.vector.scalar_tensor_tensor(
            d_t[:], sg_t[:], -2.0, hloc1_sb[:], ALU.mult, ALU.add
        )
        nc.vector.tensor_scalar(
            n_t[:], sg_t[:], 2.0, 1.0, ALU.mult, ALU.subtract
        )

        # W_hz: slabs 6-7 (DoubleRow), then the z tail in column halves
        # on independent rings (both out-DMAs dispatch concurrently)
        dr_slab(6, 3, vth8p_sb, opener=z_opener)
        filler(N_FILL)
        dr_slab(7, 3, vth8p_sb)
        for hf in range(2):
            cols = slice(hf * ZH, (hf + 1) * ZH)
            nc.scalar.activation(
                z_t[:, cols], acc[3][:, cols], AF.Sigmoid,
                scale=1.0 / WHR_SCALE,
            )
            nc.vector.tensor_mul(zd_t[:, cols], z_t[:, cols], d_t[:, cols])
            nc.vector.tensor_add(o_t[:, cols], zd_t[:, cols], n_t[:, cols])
            dma_eng = nc.sync if hf == 0 else nc.gpsimd
            dma_eng.dma_start(out[:, cols], o_t[:, cols])

    nc.compile()
    return nc


def _get_nc():
    global _CACHED_NC
    if _CACHED_NC is None:
        _CACHED_NC = _build()
    return _CACHED_NC


def make_in_maps(x, h, adj, W_hr, b_hr, W_hz, b_hz, W_hn, b_hn):
    bf = ml_dtypes.bfloat16
    fp8 = ml_dtypes.float8_e4m3fn
    fp8e3 = ml_dtypes.float8_e3m4
    x = np.asarray(x, np.float32)
    h = np.asarray(h, np.float32)
    adj = np.asarray(adj, np.float32)
    W_hr = np.asarray(W_hr, np.float32)
    W_hz = np.asarray(W_hz, np.float32)
    W_hn = np.asarray(W_hn, np.float32)
    b_hr = np.asarray(b_hr, np.float32)
    b_hz = np.asarray(b_hz, np.float32)
    b_hn = np.asarray(b_hn, np.float32)

    def pack_vt(v):
        # [B, N] -> [KC, NK, B] chunk-major
        return np.ascontiguousarray(v.T.reshape(NK, KC, B).transpose(1, 0, 2))

    def pack_vt_pairs(v):
        # [KC, NK, B] -> [KC, 2, NKP*B]: even chunks plane 0, odd plane 1
        c = pack_vt(v)
        return np.ascontiguousarray(
            c.reshape(KC, NKP, 2, B).transpose(0, 2, 1, 3).reshape(KC, 2, NKP * B)
        )

    vtx8p_packed = pack_vt_pairs(x).astype(fp8)
    vth8p_packed = pack_vt_pairs(h).astype(fp8)
    vth_packed = pack_vt(h).reshape(KC, NK * B).astype(bf)

    def pack_slabs(chunks_2d, nslabs):
        # [N, S] (contraction-major) -> [nslabs, KC, CPS, S]
        return np.ascontiguousarray(
            chunks_2d.reshape(nslabs, CPS, KC, S).transpose(0, 2, 1, 3)
        )

    in_maps = []
    for s in range(NCORES):
        rs, re = s * S, (s + 1) * S
        wdrp = np.concatenate(
            [
                pack_slabs(np.ascontiguousarray(adj[rs:re].T) * ADJ_SCALE, 2),
                pack_slabs(np.ascontiguousarray(W_hr[rs:re].T) * WHR_SCALE, 2),
                pack_slabs(np.ascontiguousarray(W_hz[rs:re].T) * WHR_SCALE, 2),
            ]
        ).astype(fp8)
        whn8p = (
            pack_slabs(np.clip(W_hn[rs:re].T * WHN_SCALE, -15.0, 15.0), 2)
            .reshape(2, KC, SLABW)
            .astype(fp8e3)
        )
        bvecp = np.concatenate(
            [b_hr[rs:re] * WHR_SCALE, b_hn[rs:re] * WHN_SCALE,
             b_hz[rs:re] * WHR_SCALE]
        )[None, :].astype(bf)
        in_maps.append(
            {
                "wdr": wdrp,
                "whn8": whn8p,
                "vtx8p": vtx8p_packed,
                "vth8p": vth8p_packed,
                "vth": vth_packed,
                "bvec": bvecp,
                "ones1": np.ones((1, B), dtype=bf),
                "hloc": np.ascontiguousarray(h[:, rs:re]),
                "hloc1": np.ascontiguousarray(h[:, rs:re] + 1.0),
                "eyez": np.eye(B, dtype=np.float32) * WHR_SCALE,
            }
        )
    return in_maps


def run(in_maps, trace=False, **kw):
    nc = _get_nc()
    return run_bass_kernel_spmd(
        nc, in_maps, core_ids=list(range(NCORES)), trace=trace, **kw
    )


def kernel(x, h, adj, W_hr, b_hr, W_hz, b_hz, W_hn, b_hn):
    in_maps = make_in_maps(x, h, adj, W_hr, b_hr, W_hz, b_hz, W_hn, b_hn)
    res = run(in_maps)
    return np.concatenate(
        [np.asarray(res.results[s]["out"]) for s in range(NCORES)], axis=1
    )


# revision 34
# speedup vs baseline: 1.0320x; 1.0205x over previous
"""AttGRU cell on 8 TRN2 NeuronCores.

Math (per reference):
    agg = einsum('ij,bj->bi', adj, x)                  # [B, N]
    r   = sigmoid(agg + h @ W_hr.T + b_hr)
    z   = sigmoid(agg + h @ W_hz.T + b_hz)
    n   = tanh(agg + r * (h @ W_hn.T + b_hn))
    out = (1 - z) * n + z * h
B=8, N=4096. Memory-bound: streaming the four [N, N] matrices dominates.

Sharding: row-shard adj/W_* over 8 cores (512 output features per core),
replicate x/h (tiny). Each core computes its 512 output columns; the host
concatenates. No collectives.

Design (all-fp8 stream, 8 MB/core vs 16 MB bf16 / 32 MB f32):
- Every weight matrix ships as 1-byte fp8, formats sized to the error
  each term can carry (l2 rel err ~1.55e-2 vs the 2e-2 gate; numpy
  simulation of the quantization reproduces the HW error to 4 digits):
  * adj (x4096), W_hr (x64), W_hz (x64): e4m3, consumed by DoubleRow
    matmuls (2 fp8 weights/PE cell -> a 1MB slab in 8 matmuls) with fp8
    x/h stationaries. r/z pre-activation errors are attenuated by
    sigmoid'; agg is a ~0.01-std additive term.
  * W_hn: e3m4 x128 (4-bit mantissa halves the quantization error; the
    n path sets output accuracy directly), consumed by normal-mode
    matmuls with *bf16* h stationary (mixed-dtype matmul is legal, so
    the h quantization error stays out of the n gate).
- The PE has a ~216ns per-matmul issue floor at N=512, independent of
  dtype and of halving N: a normal-mode 1MB slab costs 3.46us against a
  ~3us DMA slab cadence, a DoubleRow slab 1.73us. Only W_hn runs normal
  mode, so the stream is DMA-bound end to end.
- All 8 slab buffers live in SBUF simultaneously (8 MB of 24 MB): the
  HBM stream never waits on a buffer recycle. A single sync-ring queue
  sustains ~310-350 GB/s - the per-core share of the HBM stack with all
  8 cores streaming; a second ring adds no total bandwidth (measured).
- The Tile scheduler may reorder independent slab DMAs; ordering-only
  dep edges chain them so the queue transfers in consumption order.
- HAM management: the PE idles ~11us during the Tile preamble, so its
  clock gate would hold 1.2 GHz for the first ~3.4us of matmuls, and
  any >~3us idle mid-stream re-throttles it (each costs ~2x on every
  matmul for ~3.4us). Twelve full-width warmup matmuls on a memset
  scratch tile lift the gate before slab 0 lands; six narrow (N=128,
  ~90ns) fillers after each DoubleRow slab bridge the DMA cadence.
- Stream order adj -> W_hr -> W_hn -> W_hz: r_t lands mid-stream, so
  the serial r->n epilogue chain (~3us of ACT/DVE ops on [8,512] tiles)
  overlaps the W_hz slabs; d_t = (h+1) - 2*sigmoid skips a dependency
  on n_t. The tail after the last weight byte is only the z chain:
  sigmoid/mul/add in column halves, out-DMA halves on the sync and
  gpsimd rings so both dispatch concurrently.
- Biases enter PSUM via K=1 matmuls (ones[1,B].T @ b[1,S]) as group
  openers; agg folds into the z accumulator via a 64*I f32 matmul.
  tanh(u) = 2*sigmoid(2u) - 1 keeps ScalarE on a single activation
  table.

Per-core inputs (host-prepared):
  wdr  [6, 128, 16, 512] e4m3 - adj (slabs 0-1) | W_hr (2-3) | W_hz
                                (4-5), row-shard, transposed, chunk-major
  whn8 [2, 128, 8192] e3m4 - W_hn x128, chunk-major
  vtx8p [128, 2, 128] e4m3 - x.T chunk pairs (even plane 0 / odd plane 1)
  vth8p [128, 2, 128] e4m3 - h.T chunk pairs
  vth  [128, 256] bf16 - h.T per chunk (n-gate stationary)
  bvec [1, 1536] bf16  - b_hr*64 | b_hn*128 | b_hz*64 shards
  ones1 [1, 8] bf16, eyez [8, 8] f32 (=64*I)
  hloc/hloc1 [8, 512] f32 - h columns, and h+1 for the d_t shortcut
"""

from contextlib import ExitStack

import ml_dtypes
import numpy as np

import concourse.bass as bass
import concourse.tile as tile
from concourse import bacc, mybir
from concourse.bass_utils import run_bass_kernel_spmd
from concourse.tile import add_dep_helper

B = 8
N = 4096
NCORES = 8
S = N // NCORES          # 512 output cols per core
KC = 128                 # contraction chunk (PE partition dim)
NK = N // KC             # 32 chunks per gate
NKP = NK // 2            # 16 chunk pairs (DoubleRow)
CPS = 16                 # chunks per slab ([128, 16, 512] = 1MB)
SLABW = CPS * S          # 8192 flat columns per slab
NSLABS = 8               # adj(2) + W_hr(2) + W_hn(2) + W_hz(2)
WDR_SRC = {0: 0, 1: 1, 2: 2, 3: 3, 6: 4, 7: 5}  # slab -> wdr index
ZH = S // 2              # z tail computed in column halves
N_WARM = 12              # dummy matmuls to lift the PE clock gate
N_FILL = 6               # narrow fillers after each fast slab
ADJ_SCALE = 4096.0       # adj pre-scale so e4m3 doesn't flush to zero
WHR_SCALE = 64.0         # W_hr/W_hz pre-scale: N(0,1/64^2) -> N(0,1)
WHN_SCALE = 128.0        # W_hn pre-scale for e3m4 (max normal 15.5)

BF16 = mybir.dt.bfloat16
F32 = mybir.dt.float32
FP8 = mybir.dt.float8e4
FP8E3 = mybir.dt.float8e3
DR = mybir.MatmulPerfMode.DoubleRow

_CACHED_NC = None


def _build():
    nc = bacc.Bacc(
        "TRN2",
        target_bir_lowering=False,
        debug=False,
        num_devices=NCORES,
    )
    wdr = nc.dram_tensor("wdr", [6, KC, CPS, S], FP8, kind="ExternalInput")
    whn8 = nc.dram_tensor("whn8", [2, KC, SLABW], FP8E3, kind="ExternalInput")
    vtx8p = nc.dram_tensor("vtx8p", [KC, 2, NKP * B], FP8, kind="ExternalInput")
    vth8p = nc.dram_tensor("vth8p", [KC, 2, NKP * B], FP8, kind="ExternalInput")
    vth = nc.dram_tensor("vth", [KC, NK * B], BF16, kind="ExternalInput")
    bvec = nc.dram_tensor("bvec", [1, 3 * S], BF16, kind="ExternalInput")
    ones1 = nc.dram_tensor("ones1", [1, B], BF16, kind="ExternalInput")
    hloc = nc.dram_tensor("hloc", [B, S], F32, kind="ExternalInput")
    hloc1 = nc.dram_tensor("hloc1", [B, S], F32, kind="ExternalInput")
    eyez = nc.dram_tensor("eyez", [B, B], F32, kind="ExternalInput")
    out = nc.dram_tensor("out", [B, S], F32, kind="ExternalOutput")

    AF = mybir.ActivationFunctionType
    ALU = mybir.AluOpType

    with tile.TileContext(nc) as tc, ExitStack() as ctx:
        wpool = ctx.enter_context(tc.tile_pool(name="wall", bufs=NSLABS))
        cpool = ctx.enter_context(tc.tile_pool(name="const", bufs=1))
        ppool = ctx.enter_context(tc.tile_pool(name="acc", bufs=1, space="PSUM"))
        epool = ctx.enter_context(tc.tile_pool(name="epi", bufs=1))

        # PE warmup (see module docstring)
        warm_sb = cpool.tile([KC, S], BF16, tag="warm")
        nc.vector.memset(warm_sb[:], 0.0)
        warm_ps = ppool.tile([B, S], F32, tag="warmps")

        def filler(n, w=128):
            # narrow matmuls: enough PE activity to hold the HAM clock
            # gate at 8/8, at ~90ns each instead of 216ns
            for _ in range(n):
                nc.tensor.matmul(
                    warm_ps[:, :w], warm_sb[:, :B], warm_sb[:, :w],
                    start=True, stop=True,
                )

        # the warmup itself needs ~3.4us of continuous PE busy -> full width
        filler(N_WARM, w=S)

        # all consts on gpsimd SWDGE (vtx8p first - the first matmul needs
        # it); the sync ring stays clear so slab 0 issues immediately
        vtx8p_sb = cpool.tile([KC, 2, NKP * B], FP8, tag="vtx8p")
        nc.gpsimd.dma_start(vtx8p_sb[:], vtx8p[:])
        vth8p_sb = cpool.tile([KC, 2, NKP * B], FP8, tag="vth8p")
        nc.gpsimd.dma_start(vth8p_sb[:], vth8p[:])
        vth_sb = cpool.tile([KC, NK * B], BF16, tag="vth")
        nc.gpsimd.dma_start(vth_sb[:], vth[:])
        bvec_sb = cpool.tile([1, 3 * S], BF16, tag="bvec")
        nc.gpsimd.dma_start(bvec_sb[:], bvec[:])
        ones_sb = cpool.tile([1, B], BF16, tag="ones1")
        nc.gpsimd.dma_start(ones_sb[:], ones1[:])
        hloc_sb = cpool.tile([B, S], F32, tag="hloc")
        nc.gpsimd.dma_start(hloc_sb[:], hloc[:])
        hloc1_sb = cpool.tile([B, S], F32, tag="hloc1")
        nc.gpsimd.dma_start(hloc1_sb[:], hloc1[:])
        eyez_sb = cpool.tile([B, B], F32, tag="eyez")
        nc.gpsimd.dma_start(eyez_sb[:], eyez[:])

        acc = [
            ppool.tile([B, S], F32, tag=f"acc{g}", name=f"acc{g}") for g in range(4)
        ]

        # epilogue tiles, declared up front
        s_agg = epool.tile([B, S], F32, tag="sagg")
        t_r = epool.tile([B, S], F32, tag="tr")
        r_t = epool.tile([B, S], F32, tag="r")
        t_n = epool.tile([B, S], F32, tag="tn")
        t_n2 = epool.tile([B, S], F32, tag="tn2")
        sg_t = epool.tile([B, S], F32, tag="sg")
        n_t = epool.tile([B, S], F32, tag="n")
        d_t = epool.tile([B, S], F32, tag="d")
        z_t = epool.tile([B, S], F32, tag="z")
        zd_t = epool.tile([B, S], F32, tag="zd")
        o_t = epool.tile([B, S], F32, tag="o")

        def bias_open(g, lo):
            return nc.tensor.matmul(
                acc[g][:, :], ones_sb[:, :], bvec_sb[:, lo : lo + S],
                start=True, stop=False,
            )


        # the Tile scheduler is free to reorder the independent slab
        # DMAs (and did, pushing a late gate's slab ahead of an earlier
        # gate's); chain them so the queue transfers in consumption order
        last_dma = [None]

        def slab_dma(wa_flat, src_flat, first, last):
            if first:
                splits = (2048, 2048, 2048, 2048)
            elif last:
                splits = (4096, 2048, 2048)
            else:
                splits = (SLABW,)
            c0 = 0
            for w in splits:
                inst = nc.sync.dma_start(
                    wa_flat[:, c0 : c0 + w], src_flat[:, c0 : c0 + w]
                )
                if last_dma[0] is not None:
                    add_dep_helper(
                        inst.ins, last_dma[0].ins, sync=False,
                        reason="stream slabs in consumption order",
                    )
                last_dma[0] = inst
                c0 += w

        def dr_slab(sl, g, vp, opener=None):
            # one DoubleRow slab: DMA + 8 chunk-pair matmuls
            half = sl % 2
            wa = wpool.tile([KC, CPS, S], FP8, tag="wa", name=f"wa{sl}")
            slab_dma(
                wa.rearrange("p c s -> p (c s)"),
                wdr[WDR_SRC[sl]].rearrange("p c s -> p (c s)"),
                first=(sl == 0), last=(sl == NSLABS - 1),
            )
            if opener is not None:
                opener()
            for c in range(0, CPS, 2):
                kp = half * (CPS // 2) + c // 2
                nc.tensor.matmul(
                    acc[g][:, :],
                    vp[:, :, kp * B : (kp + 1) * B],
                    wa[:, c : c + 2, :],
                    start=(g == 0 and kp == 0),
                    stop=(kp == NKP - 1),
                    perf_mode=DR,
                )

        def whn_slab(sl):
            # W_hn: e3m4 moving x bf16 h stationary, normal mode. 16
            # matmuls per slab (~3.5us) against the ~3us DMA cadence -
            # the PE tracks the stream with no fillers needed, and the
            # accumulator closes right after the slab lands, keeping the
            # serial n epilogue off the tail.
            half = sl % 2
            wa = wpool.tile([KC, SLABW], FP8E3, tag="wa", name=f"wa{sl}")
            slab_dma(wa, whn8[half], first=False, last=False)
            for c in range(CPS):
                k = half * CPS + c
                if k == 0:
                    bias_open(2, S)
                nc.tensor.matmul(
                    acc[2][:, :],
                    vth_sb[:, k * B : (k + 1) * B],
                    wa[:, c * S : (c + 1) * S],
                    start=False,
                    stop=(k == NK - 1),
                )

        def z_opener():
            bias_open(3, 2 * S)
            # fold WHR_SCALE*agg into the z accumulator
            nc.tensor.matmul(
                acc[3][:, :], eyez_sb[:, :], s_agg[:, :],
                start=False, stop=False,
            )

        # ---- the stream, in explicit PE/queue order ----
        # slab order: adj(0-1), W_hr(2-3), W_hn(4-5), W_hz(6-7). W_hr
        # streams early so the serial r -> n epilogue chain (~4us of
        # ACT/DVE ops) completes during the W_hz slabs; the tail after
        # the last weight byte is only the z chain. Fillers bridge each
        # slab's ~1.7us of matmuls against the ~3us DMA cadence so HAM
        # never re-throttles.
        dr_slab(0, 0, vtx8p_sb)
        filler(N_FILL)
        dr_slab(1, 0, vtx8p_sb)
        filler(N_FILL)
        nc.vector.tensor_scalar_mul(s_agg[:], acc[0][:, :], 1.0 / ADJ_SCALE)

        # W_hr: slabs 2-3 (DoubleRow); r_t lands ~2/3 into the stream
        dr_slab(2, 1, vth8p_sb, opener=lambda: bias_open(1, 0))
        filler(N_FILL)
        dr_slab(3, 1, vth8p_sb)
        filler(N_FILL)
        nc.vector.scalar_tensor_tensor(
            t_r[:], acc[1][:, :], 1.0 / WHR_SCALE, s_agg[:],
            ALU.mult, ALU.add,
        )
        nc.scalar.activation(r_t[:], t_r[:], AF.Sigmoid)

        # W_hn: slabs 4-5 (normal mode)
        whn_slab(4)
        whn_slab(5)
        # n epilogue: starts the moment acc2 closes, overlapping the
        # W_hz stream. d_t = (h+1) - 2*sigmoid avoids waiting on n_t.
        nc.vector.tensor_mul(t_n[:], acc[2][:, :], r_t[:])
        nc.vector.scalar_tensor_tensor(
            t_n2[:], t_n[:], 1.0 / WHN_SCALE, s_agg[:], ALU.mult, ALU.add
        )
        # tanh(u) = 2*sigmoid(2u) - 1 (ACT on one table)
        nc.scalar.activation(sg_t[:], t_n2[:], AF.Sigmoid, scale=2.0)
        nc.vector.scalar_tensor_tensor(
            d_t[:], sg_t[:], -2.0, hloc1_sb[:], ALU.mult, ALU.add
        )
        nc.vector.tensor_scalar(
            n_t[:], sg_t[:], 2.0, 1.0, ALU.mult, ALU.subtract
        )

        # W_hz: slabs 6-7 (DoubleRow), then the z tail in column halves
        # on independent rings (both out-DMAs dispatch concurrently)
        dr_slab(6, 3, vth8p_sb, opener=z_opener)
        filler(N_FILL)
        dr_slab(7, 3, vth8p_sb)
        for hf in range(2):
            cols = slice(hf * ZH, (hf + 1) * ZH)
            nc.scalar.activation(
                z_t[:, cols], acc[3][:, cols], AF.Sigmoid,
                scale=1.0 / WHR_SCALE,
            )
            nc.vector.tensor_mul(zd_t[:, cols], z_t[:, cols], d_t[:, cols])
            nc.vector.tensor_add(o_t[:, cols], zd_t[:, cols], n_t[:, cols])
            dma_eng = nc.sync if hf == 0 else nc.gpsimd
            dma_eng.dma_start(out[:, cols], o_t[:, cols])

    nc.compile()
    return nc


def _get_nc():
    global _CACHED_NC
    if _CACHED_NC is None:
        _CACHED_NC = _build()
    return _CACHED_NC


def make_in_maps(x, h, adj, W_hr, b_hr, W_hz, b_hz, W_hn, b_hn):
    bf = ml_dtypes.bfloat16
    fp8 = ml_dtypes.float8_e4m3fn
    fp8e3 = ml_dtypes.float8_e3m4
    x = np.asarray(x, np.float32)
    h = np.asarray(h, np.float32)
    adj = np.asarray(adj, np.float32)
    W_hr = np.asarray(W_hr, np.float32)
    W_hz = np.asarray(W_hz, np.float32)
    W_hn = np.asarray(W_hn, np.float32)
    b_hr = np.asarray(b_hr, np.float32)
    b_hz = np.asarray(b_hz, np.float32)
    b_hn = np.asarray(b_hn, np.float32)

    def pack_vt(v):
        # [B, N] -> [KC, NK, B] chunk-major
        return np.ascontiguousarray(v.T.reshape(NK, KC, B).transpose(1, 0, 2))

    def pack_vt_pairs(v):
        # [KC, NK, B] -> [KC, 2, NKP*B]: even chunks plane 0, odd plane 1
        c = pack_vt(v)
        return np.ascontiguousarray(
            c.reshape(KC, NKP, 2, B).transpose(0, 2, 1, 3).reshape(KC, 2, NKP * B)
        )

    vtx8p_packed = pack_vt_pairs(x).astype(fp8)
    vth8p_packed = pack_vt_pairs(h).astype(fp8)
    vth_packed = pack_vt(h).reshape(KC, NK * B).astype(bf)

    def pack_slabs(chunks_2d, nslabs):
        # [N, S] (contraction-major) -> [nslabs, KC, CPS, S]
        return np.ascontiguousarray(
            chunks_2d.reshape(nslabs, CPS, KC, S).transpose(0, 2, 1, 3)
        )

    in_maps = []
    for s in range(NCORES):
        rs, re = s * S, (s + 1) * S
        wdrp = np.concatenate(
            [
                pack_slabs(np.ascontiguousarray(adj[rs:re].T) * ADJ_SCALE, 2),
                pack_slabs(np.ascontiguousarray(W_hr[rs:re].T) * WHR_SCALE, 2),
                pack_slabs(np.ascontiguousarray(W_hz[rs:re].T) * WHR_SCALE, 2),
            ]
        ).astype(fp8)
        whn8p = (
            pack_slabs(np.clip(W_hn[rs:re].T * WHN_SCALE, -15.0, 15.0), 2)
            .reshape(2, KC, SLABW)
            .astype(fp8e3)
        )
        bvecp = np.concatenate(
            [b_hr[rs:re] * WHR_SCALE, b_hn[rs:re] * WHN_SCALE,
             b_hz[rs:re] * WHR_SCALE]
        )[None, :].astype(bf)
        in_maps.append(
            {
                "wdr": wdrp,
                "whn8": whn8p,
                "vtx8p": vtx8p_packed,
                "vth8p": vth8p_packed,
                "vth": vth_packed,
                "bvec": bvecp,
                "ones1": np.ones((1, B), dtype=bf),
                "hloc": np.ascontiguousarray(h[:, rs:re]),
                "hloc1": np.ascontiguousarray(h[:, rs:re] + 1.0),
                "eyez": np.eye(B, dtype=np.float32) * WHR_SCALE,
            }
        )
    return in_maps


def run(in_maps, trace=False, **kw):
    nc = _get_nc()
    return run_bass_kernel_spmd(
        nc, in_maps, core_ids=list(range(NCORES)), trace=trace, **kw
    )


def kernel(x, h, adj, W_hr, b_hr, W_hz, b_hz, W_hn, b_hn):
    in_maps = make_in_maps(x, h, adj, W_hr, b_hr, W_hz, b_hz, W_hn, b_hn)
    res = run(in_maps)
    return np.concatenate(
        [np.asarray(res.results[s]["out"]) for s in range(NCORES)], axis=1
    )
